# revision 9
# baseline (speedup 1.0000x reference)
"""Multi-head attention Trainium2 kernel (8 NeuronCores, SPMD).

Problem: B=2, S=2048, D=1024, H=16 heads, d_k=64.
Sharding: 2 batches x 4 head-groups -> 8 cores. Core c handles batch c//4,
heads [4*(c%4), 4*(c%4)+4). Each core computes its 4 heads' Q/K/V
projections, attention, and a partial output projection (row-parallel Wo);
the host sums the 4 partials per batch (the "all-reduce" done on host).

On-device layout is feature-major ("transposed"): activations live as
[d, tokens] so that
  - projections are natural matmuls (lhsT = W.T tiles, rhs = x.T tiles),
  - scores are computed directly as S.T [k_seq, q] (k on partitions),
  - softmax sum over k (partition dim) falls out of the P@V matmul by
    augmenting V with a ones column (row 64 of the PV psum = denominators).
The 1/sqrt(d_k) scale is folded into Wq/bq on the host. Biases are applied
on the psum->SBUF copies (per-partition tensor_scalar for Q/K, a
host-replicated bias tensor for V; bo is added on the host). Matmul
operands use float32r (full-rate PE fp32); accumulation stays fp32.

Pipeline: K proj; Q proj(0); V proj interleaved with attention(qt0,pair0);
then per q-tile {Q proj(qt+1), attention(qt), out-proj(qt-1)} with
per-pair softmax-normalization chains started mid-attention.
"""

import threading
from contextlib import ExitStack

import numpy as np

import concourse.bass as bass
import concourse.tile as tile
from concourse import bacc, mybir
from concourse.bass_utils import run_bass_kernel_spmd

F32 = mybir.dt.float32
F32R = mybir.dt.float32r
AF = mybir.ActivationFunctionType

B = 2
S = 2048
D = 1024
H = 16
DK = 64
N_CORES = 8
HG = 4  # heads per core
EW = HG * DK  # 256 features per core
DT = D // 128  # 8 contraction tiles
QT = 4  # q tiles of 512
QTS = S // QT  # 512
KT = S // 128  # 16 k-seq tiles of 128
ET = D // 128  # 8 output-feature tiles


def build_program():
    """Build + compile the (single, SPMD) Bass program. Returns nc."""
    nc = bacc.Bacc("TRN2", target_bir_lowering=False, debug=False,
                   num_devices=N_CORES)

    xq = nc.dram_tensor("xq", [D, S], F32R, kind="ExternalInput").ap()
    xk = nc.dram_tensor("xk", [D, S], F32R, kind="ExternalInput").ap()
    xv = nc.dram_tensor("xv", [D, S], F32R, kind="ExternalInput").ap()
    wq = nc.dram_tensor("wq", [D, EW], F32R, kind="ExternalInput").ap()
    wk = nc.dram_tensor("wk", [D, EW], F32R, kind="ExternalInput").ap()
    wv = nc.dram_tensor("wv", [D, EW], F32R, kind="ExternalInput").ap()
    wo = nc.dram_tensor("wo", [128, 2, D], F32R, kind="ExternalInput").ap()
    bq2 = nc.dram_tensor("bq2", [128, 2], F32, kind="ExternalInput").ap()
    bk2 = nc.dram_tensor("bk2", [128, 2], F32, kind="ExternalInput").ap()
    bvr = nc.dram_tensor("bvr", [128, HG, DK], F32, kind="ExternalInput").ap()
    outT = nc.dram_tensor("outT", [D, S], F32, kind="ExternalOutput").ap()

    xq_v = xq.rearrange("(dt p) n -> p dt n", p=128)
    xk_v = xk.rearrange("(dt p) n -> p dt n", p=128)
    xv_v = xv.rearrange("(dt p) n -> p dt n", p=128)
    wq_v = wq.rearrange("(dt p) m -> p dt m", p=128)
    wk_v = wk.rearrange("(dt p) m -> p dt m", p=128)
    wv_v = wv.rearrange("(dt p) m -> p dt m", p=128)
    outT_v = outT.rearrange("(et p) n -> p et n", p=128)

    with tile.TileContext(nc) as tc, ExitStack() as ctx:
        persist = ctx.enter_context(tc.tile_pool(name="persist", bufs=1))
        wo_sb = persist.tile([128, 2, D], F32R, tag="wo_sb")
        QTi = persist.tile([128, 2, S], F32R, tag="QT")  # pair-stacked Q.T
        KTi = persist.tile([128, 2, S], F32R, tag="KT")
        VA = persist.tile([128, KT, HG, DK + 1], F32R, tag="VA")
        bq_sb = persist.tile([128, 2], F32, tag="bq_sb")
        bk_sb = persist.tile([128, 2], F32, tag="bk_sb")
        bv_sb = persist.tile([128, HG, DK], F32, tag="bv_sb")
        wpool = ctx.enter_context(tc.tile_pool(name="wts", bufs=1))
        wq_sb = wpool.tile([128, DT, EW], F32R, tag="wq_sb")
        wk_sb = wpool.tile([128, DT, EW], F32R, tag="wk_sb")
        wv_sb = wpool.tile([128, DT, EW], F32R, tag="wv_sb")
        # weights + small tensors go via SWDGE (gpsimd) so they overlap the
        # HWDGE (sync) x-tile streams
        nc.gpsimd.dma_start(wk_sb[:], wk_v[:])
        nc.gpsimd.dma_start(bq_sb[:], bq2[:])
        nc.gpsimd.dma_start(bk_sb[:], bk2[:])
        nc.gpsimd.dma_start(bv_sb[:], bvr[:])
        nc.gpsimd.memset(VA[:, :, :, DK:DK + 1].bitcast(F32), 1.0)

        xpool = ctx.enter_context(tc.tile_pool(name="xin", bufs=2))
        # single PSUM pool, statically 8 banks:
        #   tag "s": [128, 2, 512] (2 banks) x 2 bufs = 4 banks
        #   tag "o": [128, 512]    (1 bank)  x 4 bufs = 4 banks
        psp = ctx.enter_context(tc.tile_pool(name="psp", bufs=1, space="PSUM"))
        ptpool = ctx.enter_context(tc.tile_pool(name="pt", bufs=3))
        oapool = ctx.enter_context(tc.tile_pool(name="oa", bufs=2))
        rpool = ctx.enter_context(tc.tile_pool(name="rr", bufs=2))
        apool = ctx.enter_context(tc.tile_pool(name="atile", bufs=4))
        obuf = ctx.enter_context(tc.tile_pool(name="obuf", bufs=3))

        def otile(name):
            return psp.tile([128, QTS], F32, tag="o", bufs=4, name=name)

        def stile(name):
            return psp.tile([128, 2, QTS], F32, tag="s", bufs=2, name=name)

        # ---- K projection (transposed layout) ----
        for qt in range(QT):
            qs = slice(qt * QTS, (qt + 1) * QTS)
            xt = xpool.tile([128, DT, QTS], F32R, tag="xt", name="xkt")
            nc.sync.dma_start(xt[:], xk_v[:, :, qs])
            for p in range(2):
                ps = otile("psk")
                for dt in range(DT):
                    nc.tensor.matmul(
                        ps[:], wk_sb[:, dt, p * 128:(p + 1) * 128],
                        xt[:, dt, :], start=(dt == 0), stop=(dt == DT - 1))
                nc.vector.tensor_scalar_add(
                    KTi[:, p, qs], ps[:], bk_sb[:, p:p + 1])

        nc.gpsimd.dma_start(wq_sb[:], wq_v[:])

        def q_proj(qt):
            qs = slice(qt * QTS, (qt + 1) * QTS)
            xt = xpool.tile([128, DT, QTS], F32R, tag="xt", name="xqt")
            nc.sync.dma_start(xt[:], xq_v[:, :, qs])
            for p in range(2):
                ps = otile("psq")
                for dt in range(DT):
                    nc.tensor.matmul(
                        ps[:], wq_sb[:, dt, p * 128:(p + 1) * 128],
                        xt[:, dt, :], start=(dt == 0), stop=(dt == DT - 1))
                nc.vector.tensor_scalar_add(
                    QTi[:, p, qs], ps[:], bq_sb[:, p:p + 1])

        q_proj(0)
        nc.gpsimd.dma_start(wv_sb[:], wv_v[:])
        nc.gpsimd.dma_start(wo_sb[:], wo[:])

        def v_proj(st):
            ss_ = slice(st * QTS, (st + 1) * QTS)
            xt = xpool.tile([128, DT, QTS], F32R, tag="xt", name="xvt")
            nc.sync.dma_start(xt[:], xv_v[:, :, ss_])
            for ss in range(4):
                kti = st * 4 + ss
                ps = otile("psv")
                for dt in range(DT):
                    nc.tensor.matmul(
                        ps[:, 0:EW], xt[:, dt, ss * 128:(ss + 1) * 128],
                        wv_sb[:, dt, :], start=(dt == 0), stop=(dt == DT - 1))
                nc.vector.tensor_add(
                    VA[:, kti, :, 0:DK],
                    ps[:, 0:EW].rearrange("p (h d) -> p h d", h=HG),
                    bv_sb[:])

        def attn_pair_kts(qt, p, po, kts):
            qs = slice(qt * QTS, (qt + 1) * QTS)
            for kt in kts:
                ks = slice(kt * 128, (kt + 1) * 128)
                ps_s = stile("pss")
                for hh in range(2):
                    r0 = 64 * hh
                    nc.tensor.matmul(
                        ps_s[:, hh, :],
                        KTi[r0:r0 + 64, p, ks],
                        QTi[r0:r0 + 64, p, qs],
                        start=True, stop=True)
                pt_t = ptpool.tile([128, 2, QTS], F32R, tag="pt")
                nc.scalar.activation(pt_t[:], ps_s[:], AF.Exp)
                for hh in range(2):
                    h = 2 * p + hh
                    nc.tensor.matmul(
                        po[hh][0:DK + 1, :], VA[:, kt, h, :],
                        pt_t[:, hh, :],
                        start=(kt == 0), stop=(kt == KT - 1))

        def attn_finish_pair(p, po, OA_t):
            for hh in range(2):
                h = 2 * p + hh
                nc.vector.tensor_copy(
                    OA_t[0:DK + 1, h, :], po[hh][0:DK + 1, :])

        def norm_pair(p, OA_t):
            """1/l for pair p, broadcast, multiply -> a2 [128, 512]
            (head 2p on partitions 0:64, head 2p+1 on 64:128)."""
            hs = slice(2 * p, 2 * p + 2)
            r0_t = rpool.tile([1, 2, QTS], F32, tag="r0", name="r0_t")
            rb_t = rpool.tile([128, 2, QTS], F32, tag="rb", name="rb_t")
            nc.sync.dma_start(r0_t[0:1, :, :], OA_t[DK:DK + 1, hs, :])
            nc.vector.reciprocal(r0_t[0:1, :, :], r0_t[0:1, :, :])
            nc.gpsimd.partition_broadcast(rb_t[:], r0_t[0:1, :, :])
            OAs = apool.tile([128, QTS], F32, tag="oas", name="OAs", bufs=2)
            nc.sync.dma_start(OAs[DK:128, :], OA_t[0:DK, 2 * p + 1, :])
            a2 = apool.tile([128, QTS], F32R, tag="at", name="a2")
            nc.vector.tensor_mul(
                a2[0:DK, :], OA_t[0:DK, 2 * p, :], rb_t[0:DK, 0, :])
            nc.vector.tensor_mul(
                a2[DK:128, :], OAs[DK:128, :], rb_t[DK:128, 1, :])
            return a2

        def out_proj(qt, a2s):
            qs = slice(qt * QTS, (qt + 1) * QTS)
            for et in range(ET):
                ps = otile("pso")
                for p in range(2):
                    nc.tensor.matmul(
                        ps[:], wo_sb[:, p, et * 128:(et + 1) * 128],
                        a2s[p][:], start=(p == 0), stop=(p == 1))
                ot = obuf.tile([128, QTS], F32, tag="ot")
                nc.vector.tensor_copy(ot[:], ps[:])
                nc.sync.dma_start(outT_v[:, et, qs], ot[:])

        # V proj interleaved with attention(qt0, pair0); then steady-state
        # per-qt pipeline with out-proj one qt behind.
        OA_t0 = oapool.tile([128, HG, QTS], F32, tag="oa", name="OA_t0")
        po0 = [otile(f"po0_{i}") for i in range(2)]
        for st in range(QT):
            v_proj(st)
            attn_pair_kts(0, 0, po0, range(4 * st, 4 * st + 4))
        attn_finish_pair(0, po0, OA_t0)
        a2s0 = [norm_pair(0, OA_t0)]

        q_proj(1)
        po1 = [otile(f"po1_{i}") for i in range(2)]
        attn_pair_kts(0, 1, po1, range(KT))
        attn_finish_pair(1, po1, OA_t0)
        a2s0.append(norm_pair(1, OA_t0))

        pending = (0, a2s0)
        for qt in range(1, QT):
            if qt + 1 < QT:
                q_proj(qt + 1)
            OA_t = oapool.tile([128, HG, QTS], F32, tag="oa", name="OA_t")
            a2s = []
            for p in range(2):
                po = [otile(f"po{i}") for i in range(2)]
                attn_pair_kts(qt, p, po, range(KT))
                attn_finish_pair(p, po, OA_t)
                a2s.append(norm_pair(p, OA_t))
            out_proj(*pending)
            pending = (qt, a2s)
        out_proj(*pending)

    nc.compile()
    return nc


_CACHE = {}
_CACHE_LOCK = threading.Lock()


def _get_program():
    with _CACHE_LOCK:
        if "nc" not in _CACHE:
            _CACHE["nc"] = build_program()
        return _CACHE["nc"]


def _prep_inputs(q, k, v, Wq, bq, Wk, bk, Wv, bv, Wo, bo):
    """Build the 8 per-core input maps (all float32 numpy)."""
    scale = np.float32(1.0 / np.sqrt(DK))

    xs = []
    for b_i in range(B):
        xs.append((np.ascontiguousarray(q[b_i].T),
                   np.ascontiguousarray(k[b_i].T),
                   np.ascontiguousarray(v[b_i].T)))

    in_maps = []
    for c in range(N_CORES):
        b_i, g = divmod(c, HG)
        sl = slice(g * EW, (g + 1) * EW)
        wo_c = Wo[:, sl].T  # [EW, D] = [(2 pairs x 128), D]
        wo_c = np.ascontiguousarray(
            wo_c.reshape(2, 128, D).transpose(1, 0, 2))  # [128, 2, D]
        bq_c = np.ascontiguousarray(
            (bq[sl] * scale).reshape(2, 128).T)  # [128, 2]
        bk_c = np.ascontiguousarray(bk[sl].reshape(2, 128).T)
        bv_c = np.ascontiguousarray(
            np.broadcast_to(bv[sl].reshape(1, HG, DK), (128, HG, DK)))
        in_maps.append({
            "xq": xs[b_i][0], "xk": xs[b_i][1], "xv": xs[b_i][2],
            "wq": np.ascontiguousarray(Wq[sl].T * scale),
            "wk": np.ascontiguousarray(Wk[sl].T),
            "wv": np.ascontiguousarray(Wv[sl].T),
            "wo": wo_c,
            "bq2": bq_c, "bk2": bk_c, "bvr": bv_c,
        })
    return in_maps


def kernel(q, k, v, Wq, bq, Wk, bk, Wv, bv, Wo, bo):
    q = np.asarray(q, np.float32)
    k = np.asarray(k, np.float32)
    v = np.asarray(v, np.float32)
    in_maps = _prep_inputs(q, k, v,
                           np.asarray(Wq, np.float32), np.asarray(bq, np.float32),
                           np.asarray(Wk, np.float32), np.asarray(bk, np.float32),
                           np.asarray(Wv, np.float32), np.asarray(bv, np.float32),
                           np.asarray(Wo, np.float32), np.asarray(bo, np.float32))
    nc = _get_program()
    res = run_bass_kernel_spmd(nc, in_maps, core_ids=list(range(N_CORES)))
    out = np.zeros((B, S, D), np.float32)
    for c in range(N_CORES):
        b_i = c // HG
        out[b_i] += res.results[c]["outT"].T
    out += np.asarray(bo, np.float32)
    return out


# revision 10
# speedup vs baseline: 1.0648x; 1.0648x over previous
"""Multi-head attention Trainium2 kernel (8 NeuronCores, SPMD).

Problem: B=2, S=2048, D=1024, H=16 heads, d_k=64.
Sharding: 2 batches x 4 head-groups -> 8 cores. Core c handles batch c//4,
heads [4*(c%4), 4*(c%4)+4). Each core computes its 4 heads' Q/K/V
projections, attention, and a partial output projection (row-parallel Wo);
the host sums the 4 partials per batch (the "all-reduce" done on host).

On-device layout is feature-major ("transposed"): activations live as
[d, tokens] so that
  - projections are natural matmuls (lhsT = W.T tiles, rhs = x.T tiles),
  - scores are computed directly as S.T [k_seq, q] (k on partitions),
  - softmax sum over k (partition dim) falls out of the P@V matmul by
    augmenting V with a ones column (row 64 of the PV psum = denominators).
The 1/sqrt(d_k) scale is folded into Wq/bq on the host. Biases are applied
on the psum->SBUF copies (per-partition tensor_scalar for Q/K, a
host-replicated bias tensor for V; bo is added on the host). Matmul
operands use float32r (full-rate PE fp32); accumulation stays fp32.

Pipeline: K proj; Q proj(0); V proj interleaved with attention(qt0,pair0);
then per q-tile {Q proj(qt+1), attention(qt), out-proj(qt-1)} with
per-pair softmax-normalization chains started mid-attention.
"""

import threading
from contextlib import ExitStack

import numpy as np

import concourse.bass as bass
import concourse.tile as tile
from concourse import bacc, mybir
from concourse.bass_utils import run_bass_kernel_spmd

F32 = mybir.dt.float32
F32R = mybir.dt.float32r
AF = mybir.ActivationFunctionType

B = 2
S = 2048
D = 1024
H = 16
DK = 64
N_CORES = 8
HG = 4  # heads per core
EW = HG * DK  # 256 features per core
DT = D // 128  # 8 contraction tiles
QT = 4  # q tiles of 512
QTS = S // QT  # 512
KT = S // 128  # 16 k-seq tiles of 128
ET = D // 128  # 8 output-feature tiles


def build_program():
    """Build + compile the (single, SPMD) Bass program. Returns nc."""
    nc = bacc.Bacc("TRN2", target_bir_lowering=False, debug=False,
                   num_devices=N_CORES)

    xq = nc.dram_tensor("xq", [D, S], F32R, kind="ExternalInput").ap()
    xk = nc.dram_tensor("xk", [D, S], F32R, kind="ExternalInput").ap()
    xv = nc.dram_tensor("xv", [D, S], F32R, kind="ExternalInput").ap()
    wq = nc.dram_tensor("wq", [D, EW], F32R, kind="ExternalInput").ap()
    wk = nc.dram_tensor("wk", [D, EW], F32R, kind="ExternalInput").ap()
    wv = nc.dram_tensor("wv", [D, EW], F32R, kind="ExternalInput").ap()
    wo = nc.dram_tensor("wo", [128, 2, D], F32R, kind="ExternalInput").ap()
    bq2 = nc.dram_tensor("bq2", [128, 2], F32, kind="ExternalInput").ap()
    bk2 = nc.dram_tensor("bk2", [128, 2], F32, kind="ExternalInput").ap()
    bvr = nc.dram_tensor("bvr", [128, HG, DK], F32, kind="ExternalInput").ap()
    outT = nc.dram_tensor("outT", [D, S], F32, kind="ExternalOutput").ap()

    xq_v = xq.rearrange("(dt p) n -> p dt n", p=128)
    xk_v = xk.rearrange("(dt p) n -> p dt n", p=128)
    xv_v = xv.rearrange("(dt p) n -> p dt n", p=128)
    wq_v = wq.rearrange("(dt p) m -> p dt m", p=128)
    wk_v = wk.rearrange("(dt p) m -> p dt m", p=128)
    wv_v = wv.rearrange("(dt p) m -> p dt m", p=128)
    outT_v = outT.rearrange("(et p) n -> p et n", p=128)

    with tile.TileContext(nc) as tc, ExitStack() as ctx:
        persist = ctx.enter_context(tc.tile_pool(name="persist", bufs=1))
        wo_sb = persist.tile([128, 2, D], F32R, tag="wo_sb")
        QTi = persist.tile([128, 2, S], F32R, tag="QT")  # pair-stacked Q.T
        KTi = persist.tile([128, 2, S], F32R, tag="KT")
        VA = persist.tile([128, KT, HG, DK + 1], F32R, tag="VA")
        bq_sb = persist.tile([128, 2], F32, tag="bq_sb")
        bk_sb = persist.tile([128, 2], F32, tag="bk_sb")
        bv_sb = persist.tile([128, HG, DK], F32, tag="bv_sb")
        wpool = ctx.enter_context(tc.tile_pool(name="wts", bufs=1))
        wq_sb = wpool.tile([128, DT, EW], F32R, tag="wq_sb")
        wk_sb = wpool.tile([128, DT, EW], F32R, tag="wk_sb")
        wv_sb = wpool.tile([128, DT, EW], F32R, tag="wv_sb")
        nc.sync.dma_start(wk_sb[:], wk_v[:])
        nc.sync.dma_start(bq_sb[:], bq2[:])
        nc.sync.dma_start(bk_sb[:], bk2[:])
        nc.sync.dma_start(bvr_sb := bv_sb[:], bvr[:])
        nc.gpsimd.memset(VA[:, :, :, DK:DK + 1].bitcast(F32), 1.0)

        xpool = ctx.enter_context(tc.tile_pool(name="xin", bufs=2))
        # single PSUM pool, statically 8 banks:
        #   tag "s": [128, 2, 512] (2 banks) x 2 bufs = 4 banks
        #   tag "o": [128, 512]    (1 bank)  x 4 bufs = 4 banks
        psp = ctx.enter_context(tc.tile_pool(name="psp", bufs=1, space="PSUM"))
        ptpool = ctx.enter_context(tc.tile_pool(name="pt", bufs=3))
        oapool = ctx.enter_context(tc.tile_pool(name="oa", bufs=2))
        rpool = ctx.enter_context(tc.tile_pool(name="rr", bufs=2))
        apool = ctx.enter_context(tc.tile_pool(name="atile", bufs=4))
        obuf = ctx.enter_context(tc.tile_pool(name="obuf", bufs=3))

        def otile(name):
            return psp.tile([128, QTS], F32, tag="o", bufs=4, name=name)

        def stile(name):
            return psp.tile([128, 2, QTS], F32, tag="s", bufs=2, name=name)

        # ---- K projection (transposed layout) ----
        for qt in range(QT):
            qs = slice(qt * QTS, (qt + 1) * QTS)
            xt = xpool.tile([128, DT, QTS], F32R, tag="xt", name="xkt")
            nc.sync.dma_start(xt[:], xk_v[:, :, qs])
            for p in range(2):
                ps = otile("psk")
                for dt in range(DT):
                    nc.tensor.matmul(
                        ps[:], wk_sb[:, dt, p * 128:(p + 1) * 128],
                        xt[:, dt, :], start=(dt == 0), stop=(dt == DT - 1))
                nc.vector.tensor_scalar_add(
                    KTi[:, p, qs], ps[:], bk_sb[:, p:p + 1])

        nc.sync.dma_start(wq_sb[:], wq_v[:])

        def q_proj(qt):
            qs = slice(qt * QTS, (qt + 1) * QTS)
            xt = xpool.tile([128, DT, QTS], F32R, tag="xt", name="xqt")
            nc.sync.dma_start(xt[:], xq_v[:, :, qs])
            for p in range(2):
                ps = otile("psq")
                for dt in range(DT):
                    nc.tensor.matmul(
                        ps[:], wq_sb[:, dt, p * 128:(p + 1) * 128],
                        xt[:, dt, :], start=(dt == 0), stop=(dt == DT - 1))
                nc.vector.tensor_scalar_add(
                    QTi[:, p, qs], ps[:], bq_sb[:, p:p + 1])

        q_proj(0)
        nc.sync.dma_start(wv_sb[:], wv_v[:])
        nc.sync.dma_start(wo_sb[:], wo[:])

        def v_proj(st):
            ss_ = slice(st * QTS, (st + 1) * QTS)
            xt = xpool.tile([128, DT, QTS], F32R, tag="xt", name="xvt")
            nc.sync.dma_start(xt[:], xv_v[:, :, ss_])
            for ss in range(4):
                kti = st * 4 + ss
                ps = otile("psv")
                for dt in range(DT):
                    nc.tensor.matmul(
                        ps[:, 0:EW], xt[:, dt, ss * 128:(ss + 1) * 128],
                        wv_sb[:, dt, :], start=(dt == 0), stop=(dt == DT - 1))
                nc.vector.tensor_add(
                    VA[:, kti, :, 0:DK],
                    ps[:, 0:EW].rearrange("p (h d) -> p h d", h=HG),
                    bv_sb[:])

        def attn_pair_kts(qt, p, po, kts):
            qs = slice(qt * QTS, (qt + 1) * QTS)
            for kt in kts:
                ks = slice(kt * 128, (kt + 1) * 128)
                ps_s = stile("pss")
                for hh in range(2):
                    r0 = 64 * hh
                    nc.tensor.matmul(
                        ps_s[:, hh, :],
                        KTi[r0:r0 + 64, p, ks],
                        QTi[r0:r0 + 64, p, qs],
                        start=True, stop=True)
                pt_t = ptpool.tile([128, 2, QTS], F32R, tag="pt")
                nc.scalar.activation(pt_t[:], ps_s[:], AF.Exp)
                for hh in range(2):
                    h = 2 * p + hh
                    nc.tensor.matmul(
                        po[hh][0:DK + 1, :], VA[:, kt, h, :],
                        pt_t[:, hh, :],
                        start=(kt == 0), stop=(kt == KT - 1))

        def attn_finish_pair(p, po, OA_t):
            for hh in range(2):
                h = 2 * p + hh
                nc.vector.tensor_copy(
                    OA_t[0:DK + 1, h, :], po[hh][0:DK + 1, :])

        def norm_pair(p, OA_t):
            """1/l for pair p, broadcast, multiply -> a2 [128, 512]
            (head 2p on partitions 0:64, head 2p+1 on 64:128)."""
            hs = slice(2 * p, 2 * p + 2)
            r0_t = rpool.tile([1, 2, QTS], F32, tag="r0", name="r0_t")
            rb_t = rpool.tile([128, 2, QTS], F32, tag="rb", name="rb_t")
            nc.sync.dma_start(r0_t[0:1, :, :], OA_t[DK:DK + 1, hs, :])
            nc.vector.reciprocal(r0_t[0:1, :, :], r0_t[0:1, :, :])
            nc.gpsimd.partition_broadcast(rb_t[:], r0_t[0:1, :, :])
            OAs = apool.tile([128, QTS], F32, tag="oas", name="OAs", bufs=2)
            nc.sync.dma_start(OAs[DK:128, :], OA_t[0:DK, 2 * p + 1, :])
            a2 = apool.tile([128, QTS], F32R, tag="at", name="a2")
            nc.vector.tensor_mul(
                a2[0:DK, :], OA_t[0:DK, 2 * p, :], rb_t[0:DK, 0, :])
            nc.vector.tensor_mul(
                a2[DK:128, :], OAs[DK:128, :], rb_t[DK:128, 1, :])
            return a2

        def out_proj(qt, a2s):
            qs = slice(qt * QTS, (qt + 1) * QTS)
            for et in range(ET):
                ps = otile("pso")
                for p in range(2):
                    nc.tensor.matmul(
                        ps[:], wo_sb[:, p, et * 128:(et + 1) * 128],
                        a2s[p][:], start=(p == 0), stop=(p == 1))
                ot = obuf.tile([128, QTS], F32, tag="ot")
                nc.vector.tensor_copy(ot[:], ps[:])
                nc.sync.dma_start(outT_v[:, et, qs], ot[:])

        # V proj interleaved with attention(qt0, pair0); then steady-state
        # per-qt pipeline with out-proj one qt behind.
        OA_t0 = oapool.tile([128, HG, QTS], F32, tag="oa", name="OA_t0")
        po0 = [otile(f"po0_{i}") for i in range(2)]
        for st in range(QT):
            v_proj(st)
            attn_pair_kts(0, 0, po0, range(4 * st, 4 * st + 4))
        attn_finish_pair(0, po0, OA_t0)
        a2s0 = [norm_pair(0, OA_t0)]

        q_proj(1)
        po1 = [otile(f"po1_{i}") for i in range(2)]
        attn_pair_kts(0, 1, po1, range(KT))
        attn_finish_pair(1, po1, OA_t0)
        a2s0.append(norm_pair(1, OA_t0))

        pending = (0, a2s0)
        for qt in range(1, QT):
            if qt + 1 < QT:
                q_proj(qt + 1)
            OA_t = oapool.tile([128, HG, QTS], F32, tag="oa", name="OA_t")
            a2s = []
            for p in range(2):
                po = [otile(f"po{i}") for i in range(2)]
                attn_pair_kts(qt, p, po, range(KT))
                attn_finish_pair(p, po, OA_t)
                a2s.append(norm_pair(p, OA_t))
            out_proj(*pending)
            pending = (qt, a2s)
        out_proj(*pending)

    nc.compile()
    return nc


_CACHE = {}
_CACHE_LOCK = threading.Lock()


def _get_program():
    with _CACHE_LOCK:
        if "nc" not in _CACHE:
            _CACHE["nc"] = build_program()
        return _CACHE["nc"]


def _prep_inputs(q, k, v, Wq, bq, Wk, bk, Wv, bv, Wo, bo):
    """Build the 8 per-core input maps (all float32 numpy)."""
    scale = np.float32(1.0 / np.sqrt(DK))

    xs = []
    for b_i in range(B):
        xs.append((np.ascontiguousarray(q[b_i].T),
                   np.ascontiguousarray(k[b_i].T),
                   np.ascontiguousarray(v[b_i].T)))

    in_maps = []
    for c in range(N_CORES):
        b_i, g = divmod(c, HG)
        sl = slice(g * EW, (g + 1) * EW)
        wo_c = Wo[:, sl].T  # [EW, D] = [(2 pairs x 128), D]
        wo_c = np.ascontiguousarray(
            wo_c.reshape(2, 128, D).transpose(1, 0, 2))  # [128, 2, D]
        bq_c = np.ascontiguousarray(
            (bq[sl] * scale).reshape(2, 128).T)  # [128, 2]
        bk_c = np.ascontiguousarray(bk[sl].reshape(2, 128).T)
        bv_c = np.ascontiguousarray(
            np.broadcast_to(bv[sl].reshape(1, HG, DK), (128, HG, DK)))
        in_maps.append({
            "xq": xs[b_i][0], "xk": xs[b_i][1], "xv": xs[b_i][2],
            "wq": np.ascontiguousarray(Wq[sl].T * scale),
            "wk": np.ascontiguousarray(Wk[sl].T),
            "wv": np.ascontiguousarray(Wv[sl].T),
            "wo": wo_c,
            "bq2": bq_c, "bk2": bk_c, "bvr": bv_c,
        })
    return in_maps


def kernel(q, k, v, Wq, bq, Wk, bk, Wv, bv, Wo, bo):
    q = np.asarray(q, np.float32)
    k = np.asarray(k, np.float32)
    v = np.asarray(v, np.float32)
    in_maps = _prep_inputs(q, k, v,
                           np.asarray(Wq, np.float32), np.asarray(bq, np.float32),
                           np.asarray(Wk, np.float32), np.asarray(bk, np.float32),
                           np.asarray(Wv, np.float32), np.asarray(bv, np.float32),
                           np.asarray(Wo, np.float32), np.asarray(bo, np.float32))
    nc = _get_program()
    res = run_bass_kernel_spmd(nc, in_maps, core_ids=list(range(N_CORES)))
    out = np.zeros((B, S, D), np.float32)
    for c in range(N_CORES):
        b_i = c // HG
        out[b_i] += res.results[c]["outT"].T
    out += np.asarray(bo, np.float32)
    return out


# revision 11
# speedup vs baseline: 1.0911x; 1.0247x over previous
"""Multi-head attention Trainium2 kernel (8 NeuronCores, SPMD).

Problem: B=2, S=2048, D=1024, H=16 heads, d_k=64.
Sharding: 2 batches x 4 head-groups -> 8 cores. Core c handles batch c//4,
heads [4*(c%4), 4*(c%4)+4). Each core computes its 4 heads' Q/K/V
projections, attention, and a partial output projection (row-parallel Wo);
the host sums the 4 partials per batch (the "all-reduce" done on host).

On-device layout is feature-major ("transposed"): activations live as
[d, tokens] so that
  - projections are natural matmuls (lhsT = W.T tiles, rhs = x.T tiles),
  - scores are computed directly as S.T [k_seq, q] (k on partitions),
  - softmax sum over k (partition dim) falls out of the P@V matmul by
    augmenting V with a ones column (row 64 of the PV psum = denominators).
The 1/sqrt(d_k) scale is folded into Wq/bq on the host. Biases are applied
on the psum->SBUF copies (per-partition tensor_scalar for Q/K, a
host-replicated bias tensor for V; bo is added on the host). Matmul
operands use float32r (full-rate PE fp32); accumulation stays fp32.

Pipeline: K proj; Q proj(0); V proj interleaved with attention(qt0,pair0);
then per q-tile {Q proj(qt+1), attention(qt), out-proj(qt-1)} with
per-pair softmax-normalization chains started mid-attention.
"""

import threading
from contextlib import ExitStack

import numpy as np

import concourse.bass as bass
import concourse.tile as tile
from concourse import bacc, mybir
from concourse.bass_utils import run_bass_kernel_spmd

F32 = mybir.dt.float32
F32R = mybir.dt.float32r
AF = mybir.ActivationFunctionType

B = 2
S = 2048
D = 1024
H = 16
DK = 64
N_CORES = 8
HG = 4  # heads per core
EW = HG * DK  # 256 features per core
DT = D // 128  # 8 contraction tiles
QT = 4  # q tiles of 512
QTS = S // QT  # 512
KT = S // 128  # 16 k-seq tiles of 128
ET = D // 128  # 8 output-feature tiles


def build_program():
    """Build + compile the (single, SPMD) Bass program. Returns nc."""
    nc = bacc.Bacc("TRN2", target_bir_lowering=False, debug=False,
                   num_devices=N_CORES)

    xq = nc.dram_tensor("xq", [D, S], F32R, kind="ExternalInput").ap()
    xk = nc.dram_tensor("xk", [D, S], F32R, kind="ExternalInput").ap()
    xv = nc.dram_tensor("xv", [D, S], F32R, kind="ExternalInput").ap()
    wq = nc.dram_tensor("wq", [D, EW], F32R, kind="ExternalInput").ap()
    wk = nc.dram_tensor("wk", [D, EW], F32R, kind="ExternalInput").ap()
    wv = nc.dram_tensor("wv", [D, EW], F32R, kind="ExternalInput").ap()
    wo = nc.dram_tensor("wo", [128, 2, D], F32R, kind="ExternalInput").ap()
    bq2 = nc.dram_tensor("bq2", [128, 2], F32, kind="ExternalInput").ap()
    bk2 = nc.dram_tensor("bk2", [128, 2], F32, kind="ExternalInput").ap()
    bvr = nc.dram_tensor("bvr", [128, HG, DK], F32, kind="ExternalInput").ap()
    outT = nc.dram_tensor("outT", [D, S], F32, kind="ExternalOutput").ap()

    xq_v = xq.rearrange("(dt p) n -> p dt n", p=128)
    xk_v = xk.rearrange("(dt p) n -> p dt n", p=128)
    xv_v = xv.rearrange("(dt p) n -> p dt n", p=128)
    wq_v = wq.rearrange("(dt p) m -> p dt m", p=128)
    wk_v = wk.rearrange("(dt p) m -> p dt m", p=128)
    wv_v = wv.rearrange("(dt p) m -> p dt m", p=128)
    outT_v = outT.rearrange("(et p) n -> p et n", p=128)

    with tile.TileContext(nc) as tc, ExitStack() as ctx:
        persist = ctx.enter_context(tc.tile_pool(name="persist", bufs=1))
        wo_sb = persist.tile([128, 2, D], F32R, tag="wo_sb")
        QTi = persist.tile([128, 2, S], F32R, tag="QT")  # pair-stacked Q.T
        KTi = persist.tile([128, 2, S], F32R, tag="KT")
        VA = persist.tile([128, KT, HG, DK + 1], F32R, tag="VA")
        bq_sb = persist.tile([128, 2], F32, tag="bq_sb")
        bk_sb = persist.tile([128, 2], F32, tag="bk_sb")
        bv_sb = persist.tile([128, HG, DK], F32, tag="bv_sb")
        wpool = ctx.enter_context(tc.tile_pool(name="wts", bufs=1))
        wq_sb = wpool.tile([128, DT, EW], F32R, tag="wq_sb")
        wk_sb = wpool.tile([128, DT, EW], F32R, tag="wk_sb")
        wv_sb = wpool.tile([128, DT, EW], F32R, tag="wv_sb")
        nc.sync.dma_start(wk_sb[:], wk_v[:])
        nc.gpsimd.memset(VA[:, :, :, DK:DK + 1].bitcast(F32), 1.0)

        xpool = ctx.enter_context(tc.tile_pool(name="xin", bufs=3))
        # single PSUM pool, statically 8 banks:
        #   tag "s": [128, 2, 512] (2 banks) x 2 bufs = 4 banks
        #   tag "o": [128, 512]    (1 bank)  x 4 bufs = 4 banks
        psp = ctx.enter_context(tc.tile_pool(name="psp", bufs=1, space="PSUM"))
        ptpool = ctx.enter_context(tc.tile_pool(name="pt", bufs=3))
        oapool = ctx.enter_context(tc.tile_pool(name="oa", bufs=2))
        rpool = ctx.enter_context(tc.tile_pool(name="rr", bufs=2))
        apool = ctx.enter_context(tc.tile_pool(name="atile", bufs=4))
        obuf = ctx.enter_context(tc.tile_pool(name="obuf", bufs=3))

        def otile(name):
            return psp.tile([128, QTS], F32, tag="o", bufs=4, name=name)

        def stile(name):
            return psp.tile([128, 2, QTS], F32, tag="s", bufs=2, name=name)

        # ---- K projection (transposed layout) ----
        for qt in range(QT):
            qs = slice(qt * QTS, (qt + 1) * QTS)
            xt = xpool.tile([128, DT, QTS], F32R, tag="xt", name="xkt")
            nc.sync.dma_start(xt[:], xk_v[:, :, qs])
            if qt == 0:
                nc.sync.dma_start(bk_sb[:], bk2[:])
                nc.sync.dma_start(bq_sb[:], bq2[:])
                nc.sync.dma_start(bv_sb[:], bvr[:])
            for p in range(2):
                ps = otile("psk")
                for dt in range(DT):
                    nc.tensor.matmul(
                        ps[:], wk_sb[:, dt, p * 128:(p + 1) * 128],
                        xt[:, dt, :], start=(dt == 0), stop=(dt == DT - 1))
                nc.vector.tensor_scalar_add(
                    KTi[:, p, qs], ps[:], bk_sb[:, p:p + 1])

        nc.sync.dma_start(wq_sb[:], wq_v[:])

        def q_proj(qt):
            qs = slice(qt * QTS, (qt + 1) * QTS)
            xt = xpool.tile([128, DT, QTS], F32R, tag="xt", name="xqt")
            nc.sync.dma_start(xt[:], xq_v[:, :, qs])
            for p in range(2):
                ps = otile("psq")
                for dt in range(DT):
                    nc.tensor.matmul(
                        ps[:], wq_sb[:, dt, p * 128:(p + 1) * 128],
                        xt[:, dt, :], start=(dt == 0), stop=(dt == DT - 1))
                nc.vector.tensor_scalar_add(
                    QTi[:, p, qs], ps[:], bq_sb[:, p:p + 1])

        q_proj(0)
        nc.sync.dma_start(wv_sb[:], wv_v[:])
        nc.sync.dma_start(wo_sb[:], wo[:])

        def v_proj(st):
            ss_ = slice(st * QTS, (st + 1) * QTS)
            xt = xpool.tile([128, DT, QTS], F32R, tag="xt", name="xvt")
            nc.sync.dma_start(xt[:], xv_v[:, :, ss_])
            for ss in range(4):
                kti = st * 4 + ss
                ps = otile("psv")
                for dt in range(DT):
                    nc.tensor.matmul(
                        ps[:, 0:EW], xt[:, dt, ss * 128:(ss + 1) * 128],
                        wv_sb[:, dt, :], start=(dt == 0), stop=(dt == DT - 1))
                nc.vector.tensor_add(
                    VA[:, kti, :, 0:DK],
                    ps[:, 0:EW].rearrange("p (h d) -> p h d", h=HG),
                    bv_sb[:])

        def attn_pair_kts(qt, p, po, kts):
            qs = slice(qt * QTS, (qt + 1) * QTS)
            for kt in kts:
                ks = slice(kt * 128, (kt + 1) * 128)
                ps_s = stile("pss")
                for hh in range(2):
                    r0 = 64 * hh
                    nc.tensor.matmul(
                        ps_s[:, hh, :],
                        KTi[r0:r0 + 64, p, ks],
                        QTi[r0:r0 + 64, p, qs],
                        start=True, stop=True)
                pt_t = ptpool.tile([128, 2, QTS], F32R, tag="pt")
                nc.scalar.activation(pt_t[:], ps_s[:], AF.Exp)
                for hh in range(2):
                    h = 2 * p + hh
                    nc.tensor.matmul(
                        po[hh][0:DK + 1, :], VA[:, kt, h, :],
                        pt_t[:, hh, :],
                        start=(kt == 0), stop=(kt == KT - 1))

        def attn_finish_pair(p, po, OA_t):
            for hh in range(2):
                h = 2 * p + hh
                nc.vector.tensor_copy(
                    OA_t[0:DK + 1, h, :], po[hh][0:DK + 1, :])

        def norm_pair(p, OA_t):
            """1/l for pair p, broadcast, multiply -> a2 [128, 512]
            (head 2p on partitions 0:64, head 2p+1 on 64:128)."""
            hs = slice(2 * p, 2 * p + 2)
            r0_t = rpool.tile([1, 2, QTS], F32, tag="r0", name="r0_t")
            rb_t = rpool.tile([128, 2, QTS], F32, tag="rb", name="rb_t")
            nc.sync.dma_start(r0_t[0:1, :, :], OA_t[DK:DK + 1, hs, :])
            nc.vector.reciprocal(r0_t[0:1, :, :], r0_t[0:1, :, :])
            nc.gpsimd.partition_broadcast(rb_t[:], r0_t[0:1, :, :])
            OAs = apool.tile([128, QTS], F32, tag="oas", name="OAs", bufs=2)
            nc.sync.dma_start(OAs[DK:128, :], OA_t[0:DK, 2 * p + 1, :])
            a2 = apool.tile([128, QTS], F32R, tag="at", name="a2")
            nc.vector.tensor_mul(
                a2[0:DK, :], OA_t[0:DK, 2 * p, :], rb_t[0:DK, 0, :])
            nc.vector.tensor_mul(
                a2[DK:128, :], OAs[DK:128, :], rb_t[DK:128, 1, :])
            return a2

        def out_proj(qt, a2s):
            qs = slice(qt * QTS, (qt + 1) * QTS)
            for et in range(ET):
                ps = otile("pso")
                for p in range(2):
                    nc.tensor.matmul(
                        ps[:], wo_sb[:, p, et * 128:(et + 1) * 128],
                        a2s[p][:], start=(p == 0), stop=(p == 1))
                ot = obuf.tile([128, QTS], F32, tag="ot")
                nc.vector.tensor_copy(ot[:], ps[:])
                nc.sync.dma_start(outT_v[:, et, qs], ot[:])

        # V proj interleaved with attention(qt0, pair0); then steady-state
        # per-qt pipeline with out-proj one qt behind.
        OA_t0 = oapool.tile([128, HG, QTS], F32, tag="oa", name="OA_t0")
        po0 = [otile(f"po0_{i}") for i in range(2)]
        for st in range(QT):
            v_proj(st)
            attn_pair_kts(0, 0, po0, range(4 * st, 4 * st + 4))
        attn_finish_pair(0, po0, OA_t0)
        a2s0 = [norm_pair(0, OA_t0)]

        q_proj(1)
        po1 = [otile(f"po1_{i}") for i in range(2)]
        attn_pair_kts(0, 1, po1, range(KT))
        attn_finish_pair(1, po1, OA_t0)
        a2s0.append(norm_pair(1, OA_t0))

        pending = (0, a2s0)
        for qt in range(1, QT):
            if qt + 1 < QT:
                q_proj(qt + 1)
            OA_t = oapool.tile([128, HG, QTS], F32, tag="oa", name="OA_t")
            a2s = []
            for p in range(2):
                po = [otile(f"po{i}") for i in range(2)]
                attn_pair_kts(qt, p, po, range(KT))
                attn_finish_pair(p, po, OA_t)
                a2s.append(norm_pair(p, OA_t))
            out_proj(*pending)
            pending = (qt, a2s)
        out_proj(*pending)

    nc.compile()
    return nc


_CACHE = {}
_CACHE_LOCK = threading.Lock()


def _get_program():
    with _CACHE_LOCK:
        if "nc" not in _CACHE:
            _CACHE["nc"] = build_program()
        return _CACHE["nc"]


def _prep_inputs(q, k, v, Wq, bq, Wk, bk, Wv, bv, Wo, bo):
    """Build the 8 per-core input maps (all float32 numpy)."""
    scale = np.float32(1.0 / np.sqrt(DK))

    xs = []
    for b_i in range(B):
        xs.append((np.ascontiguousarray(q[b_i].T),
                   np.ascontiguousarray(k[b_i].T),
                   np.ascontiguousarray(v[b_i].T)))

    in_maps = []
    for c in range(N_CORES):
        b_i, g = divmod(c, HG)
        sl = slice(g * EW, (g + 1) * EW)
        wo_c = Wo[:, sl].T  # [EW, D] = [(2 pairs x 128), D]
        wo_c = np.ascontiguousarray(
            wo_c.reshape(2, 128, D).transpose(1, 0, 2))  # [128, 2, D]
        bq_c = np.ascontiguousarray(
            (bq[sl] * scale).reshape(2, 128).T)  # [128, 2]
        bk_c = np.ascontiguousarray(bk[sl].reshape(2, 128).T)
        bv_c = np.ascontiguousarray(
            np.broadcast_to(bv[sl].reshape(1, HG, DK), (128, HG, DK)))
        in_maps.append({
            "xq": xs[b_i][0], "xk": xs[b_i][1], "xv": xs[b_i][2],
            "wq": np.ascontiguousarray(Wq[sl].T * scale),
            "wk": np.ascontiguousarray(Wk[sl].T),
            "wv": np.ascontiguousarray(Wv[sl].T),
            "wo": wo_c,
            "bq2": bq_c, "bk2": bk_c, "bvr": bv_c,
        })
    return in_maps


def kernel(q, k, v, Wq, bq, Wk, bk, Wv, bv, Wo, bo):
    q = np.asarray(q, np.float32)
    k = np.asarray(k, np.float32)
    v = np.asarray(v, np.float32)
    in_maps = _prep_inputs(q, k, v,
                           np.asarray(Wq, np.float32), np.asarray(bq, np.float32),
                           np.asarray(Wk, np.float32), np.asarray(bk, np.float32),
                           np.asarray(Wv, np.float32), np.asarray(bv, np.float32),
                           np.asarray(Wo, np.float32), np.asarray(bo, np.float32))
    nc = _get_program()
    res = run_bass_kernel_spmd(nc, in_maps, core_ids=list(range(N_CORES)))
    out = np.zeros((B, S, D), np.float32)
    for c in range(N_CORES):
        b_i = c // HG
        out[b_i] += res.results[c]["outT"].T
    out += np.asarray(bo, np.float32)
    return out


# revision 12
# speedup vs baseline: 1.1484x; 1.0524x over previous
"""Multi-head attention Trainium2 kernel (8 NeuronCores, SPMD).

Problem: B=2, S=2048, D=1024, H=16 heads, d_k=64.
Sharding: 2 batches x 4 head-groups -> 8 cores. Core c handles batch c//4,
heads [4*(c%4), 4*(c%4)+4). Each core computes its 4 heads' Q/K/V
projections, attention, and a partial output projection (row-parallel Wo);
the host sums the 4 partials per batch (the "all-reduce" done on host).

On-device layout is feature-major ("transposed"): activations live as
[d, tokens] so that
  - projections are natural matmuls (lhsT = W.T tiles, rhs = x.T tiles),
  - scores are computed directly as S.T [k_seq, q] (k on partitions),
  - softmax sum over k (partition dim) falls out of the P@V matmul by
    augmenting V with a ones column (row 64 of the PV psum = denominators).
The 1/sqrt(d_k) scale is folded into Wq/bq on the host. Biases are applied
on the psum->SBUF copies (per-partition tensor_scalar for Q/K, a
host-replicated bias tensor for V; bo is added on the host). Matmul
operands use float32r (full-rate PE fp32); accumulation stays fp32.

Pipeline: K proj; Q proj(0); V proj interleaved with attention(qt0,pair0);
then per q-tile {Q proj(qt+1), attention(qt), out-proj(qt-1)} with
per-pair softmax-normalization chains started mid-attention.
"""

import threading
from contextlib import ExitStack

import numpy as np

import concourse.bass as bass
import concourse.tile as tile
from concourse import bacc, mybir
from concourse.bass_utils import run_bass_kernel_spmd

F32 = mybir.dt.float32
F32R = mybir.dt.float32r
AF = mybir.ActivationFunctionType

B = 2
S = 2048
D = 1024
H = 16
DK = 64
N_CORES = 8
HG = 4  # heads per core
EW = HG * DK  # 256 features per core
DT = D // 128  # 8 contraction tiles
QT = 4  # q tiles of 512
QTS = S // QT  # 512
KT = S // 128  # 16 k-seq tiles of 128
ET = D // 128  # 8 output-feature tiles


def build_program():
    """Build + compile the (single, SPMD) Bass program. Returns nc."""
    nc = bacc.Bacc("TRN2", target_bir_lowering=False, debug=False,
                   num_devices=N_CORES)

    xq = nc.dram_tensor("xq", [D, S], F32R, kind="ExternalInput").ap()
    xk = nc.dram_tensor("xk", [D, S], F32R, kind="ExternalInput").ap()
    xv = nc.dram_tensor("xv", [D, S], F32R, kind="ExternalInput").ap()
    wq = nc.dram_tensor("wq", [D, EW], F32R, kind="ExternalInput").ap()
    wk = nc.dram_tensor("wk", [D, EW], F32R, kind="ExternalInput").ap()
    wv = nc.dram_tensor("wv", [D, EW], F32R, kind="ExternalInput").ap()
    wo = nc.dram_tensor("wo", [128, 2, D], F32R, kind="ExternalInput").ap()
    bq2 = nc.dram_tensor("bq2", [128, 2], F32, kind="ExternalInput").ap()
    bk2 = nc.dram_tensor("bk2", [128, 2], F32, kind="ExternalInput").ap()
    bvr = nc.dram_tensor("bvr", [128, HG, DK], F32, kind="ExternalInput").ap()
    outT = nc.dram_tensor("outT", [D, S], F32, kind="ExternalOutput").ap()

    xq_v = xq.rearrange("(dt p) n -> p dt n", p=128)
    xk_v = xk.rearrange("(dt p) n -> p dt n", p=128)
    xv_v = xv.rearrange("(dt p) n -> p dt n", p=128)
    wq_v = wq.rearrange("(dt p) m -> p dt m", p=128)
    wk_v = wk.rearrange("(dt p) m -> p dt m", p=128)
    wv_v = wv.rearrange("(dt p) m -> p dt m", p=128)
    outT_v = outT.rearrange("(et p) n -> p et n", p=128)

    with tile.TileContext(nc) as tc, ExitStack() as ctx:
        persist = ctx.enter_context(tc.tile_pool(name="persist", bufs=1))
        wo_sb = persist.tile([128, 2, D], F32R, tag="wo_sb")
        QTi = persist.tile([128, 2, S], F32R, tag="QT")  # pair-stacked Q.T
        KTi = persist.tile([128, 2, S], F32R, tag="KT")
        VA = persist.tile([128, KT, HG, DK + 1], F32R, tag="VA")
        bq_sb = persist.tile([128, 2], F32, tag="bq_sb")
        bk_sb = persist.tile([128, 2], F32, tag="bk_sb")
        bv_sb = persist.tile([128, HG, DK], F32, tag="bv_sb")
        wpool = ctx.enter_context(tc.tile_pool(name="wts", bufs=1))
        wq_sb = wpool.tile([128, DT, EW], F32R, tag="wq_sb")
        wk_sb = wpool.tile([128, DT, EW], F32R, tag="wk_sb")
        wv_sb = wpool.tile([128, DT, EW], F32R, tag="wv_sb")
        nc.sync.dma_start(wk_sb[:], wk_v[:])
        nc.gpsimd.memset(VA[:, :, :, DK:DK + 1].bitcast(F32), 1.0)

        xpool = ctx.enter_context(tc.tile_pool(name="xin", bufs=3))
        # single PSUM pool, statically 8 banks:
        #   tag "s": [128, 2, 512] (2 banks) x 2 bufs = 4 banks
        #   tag "o": [128, 512]    (1 bank)  x 4 bufs = 4 banks
        psp = ctx.enter_context(tc.tile_pool(name="psp", bufs=1, space="PSUM"))
        ptpool = ctx.enter_context(tc.tile_pool(name="pt", bufs=3))
        oapool = ctx.enter_context(tc.tile_pool(name="oa", bufs=2))
        rpool = ctx.enter_context(tc.tile_pool(name="rr", bufs=2))
        apool = ctx.enter_context(tc.tile_pool(name="atile", bufs=4))
        obuf = ctx.enter_context(tc.tile_pool(name="obuf", bufs=3))

        def otile(name):
            return psp.tile([128, QTS], F32, tag="o", bufs=4, name=name)

        def stile(name):
            return psp.tile([128, 2, QTS], F32, tag="s", bufs=2, name=name)

        def k_proj(qt):
            qs = slice(qt * QTS, (qt + 1) * QTS)
            xt = xpool.tile([128, DT, QTS], F32R, tag="xt", name="xkt")
            nc.sync.dma_start(xt[:], xk_v[:, :, qs])
            if qt == 0:
                nc.sync.dma_start(bk_sb[:], bk2[:])
                nc.sync.dma_start(bq_sb[:], bq2[:])
                nc.sync.dma_start(bv_sb[:], bvr[:])
            for p in range(2):
                ps = otile("psk")
                for dt in range(DT):
                    nc.tensor.matmul(
                        ps[:], wk_sb[:, dt, p * 128:(p + 1) * 128],
                        xt[:, dt, :], start=(dt == 0), stop=(dt == DT - 1))
                nc.vector.tensor_scalar_add(
                    KTi[:, p, qs], ps[:], bk_sb[:, p:p + 1])

        k_proj(0)
        nc.sync.dma_start(wq_sb[:], wq_v[:])

        def q_proj(qt):
            qs = slice(qt * QTS, (qt + 1) * QTS)
            xt = xpool.tile([128, DT, QTS], F32R, tag="xt", name="xqt")
            nc.sync.dma_start(xt[:], xq_v[:, :, qs])
            for p in range(2):
                ps = otile("psq")
                for dt in range(DT):
                    nc.tensor.matmul(
                        ps[:], wq_sb[:, dt, p * 128:(p + 1) * 128],
                        xt[:, dt, :], start=(dt == 0), stop=(dt == DT - 1))
                nc.vector.tensor_scalar_add(
                    QTi[:, p, qs], ps[:], bq_sb[:, p:p + 1])

        q_proj(0)
        nc.sync.dma_start(wv_sb[:], wv_v[:])

        def v_proj(st):
            ss_ = slice(st * QTS, (st + 1) * QTS)
            xt = xpool.tile([128, DT, QTS], F32R, tag="xt", name="xvt")
            nc.sync.dma_start(xt[:], xv_v[:, :, ss_])
            for ss in range(4):
                kti = st * 4 + ss
                ps = otile("psv")
                for dt in range(DT):
                    nc.tensor.matmul(
                        ps[:, 0:EW], xt[:, dt, ss * 128:(ss + 1) * 128],
                        wv_sb[:, dt, :], start=(dt == 0), stop=(dt == DT - 1))
                nc.vector.tensor_add(
                    VA[:, kti, :, 0:DK],
                    ps[:, 0:EW].rearrange("p (h d) -> p h d", h=HG),
                    bv_sb[:])

        def attn_pair_kts(qt, p, po, kts):
            qs = slice(qt * QTS, (qt + 1) * QTS)
            for kt in kts:
                ks = slice(kt * 128, (kt + 1) * 128)
                ps_s = stile("pss")
                for hh in range(2):
                    r0 = 64 * hh
                    nc.tensor.matmul(
                        ps_s[:, hh, :],
                        KTi[r0:r0 + 64, p, ks],
                        QTi[r0:r0 + 64, p, qs],
                        start=True, stop=True)
                pt_t = ptpool.tile([128, 2, QTS], F32R, tag="pt")
                nc.scalar.activation(pt_t[:], ps_s[:], AF.Exp)
                for hh in range(2):
                    h = 2 * p + hh
                    nc.tensor.matmul(
                        po[hh][0:DK + 1, :], VA[:, kt, h, :],
                        pt_t[:, hh, :],
                        start=(kt == 0), stop=(kt == KT - 1))

        def attn_finish_pair(p, po, OA_t):
            for hh in range(2):
                h = 2 * p + hh
                nc.vector.tensor_copy(
                    OA_t[0:DK + 1, h, :], po[hh][0:DK + 1, :])

        def norm_pair(p, OA_t):
            """1/l for pair p, broadcast, multiply -> a2 [128, 512]
            (head 2p on partitions 0:64, head 2p+1 on 64:128)."""
            hs = slice(2 * p, 2 * p + 2)
            r0_t = rpool.tile([1, 2, QTS], F32, tag="r0", name="r0_t")
            rb_t = rpool.tile([128, 2, QTS], F32, tag="rb", name="rb_t")
            nc.sync.dma_start(r0_t[0:1, :, :], OA_t[DK:DK + 1, hs, :])
            nc.vector.reciprocal(r0_t[0:1, :, :], r0_t[0:1, :, :])
            nc.gpsimd.partition_broadcast(rb_t[:], r0_t[0:1, :, :])
            OAs = apool.tile([128, QTS], F32, tag="oas", name="OAs", bufs=2)
            nc.sync.dma_start(OAs[DK:128, :], OA_t[0:DK, 2 * p + 1, :])
            a2 = apool.tile([128, QTS], F32R, tag="at", name="a2")
            nc.vector.tensor_mul(
                a2[0:DK, :], OA_t[0:DK, 2 * p, :], rb_t[0:DK, 0, :])
            nc.vector.tensor_mul(
                a2[DK:128, :], OAs[DK:128, :], rb_t[DK:128, 1, :])
            return a2

        def out_proj(qt, a2s):
            qs = slice(qt * QTS, (qt + 1) * QTS)
            for et in range(ET):
                ps = otile("pso")
                for p in range(2):
                    nc.tensor.matmul(
                        ps[:], wo_sb[:, p, et * 128:(et + 1) * 128],
                        a2s[p][:], start=(p == 0), stop=(p == 1))
                ot = obuf.tile([128, QTS], F32, tag="ot")
                nc.vector.tensor_copy(ot[:], ps[:])
                nc.sync.dma_start(outT_v[:, et, qs], ot[:])

        # K/V proj chunks interleaved with attention(qt0, pair0); then
        # steady-state per-qt pipeline with out-proj one qt behind.
        OA_t0 = oapool.tile([128, HG, QTS], F32, tag="oa", name="OA_t0")
        po0 = [otile(f"po0_{i}") for i in range(2)]
        v_proj(0)
        nc.sync.dma_start(wo_sb[:], wo[:])
        attn_pair_kts(0, 0, po0, range(0, 4))
        for st in range(1, QT):
            k_proj(st)
            v_proj(st)
            attn_pair_kts(0, 0, po0, range(4 * st, 4 * st + 4))
        attn_finish_pair(0, po0, OA_t0)
        a2s0 = [norm_pair(0, OA_t0)]

        q_proj(1)
        po1 = [otile(f"po1_{i}") for i in range(2)]
        attn_pair_kts(0, 1, po1, range(KT))
        attn_finish_pair(1, po1, OA_t0)
        a2s0.append(norm_pair(1, OA_t0))

        pending = (0, a2s0)
        for qt in range(1, QT):
            if qt + 1 < QT:
                q_proj(qt + 1)
            OA_t = oapool.tile([128, HG, QTS], F32, tag="oa", name="OA_t")
            a2s = []
            for p in range(2):
                po = [otile(f"po{i}") for i in range(2)]
                attn_pair_kts(qt, p, po, range(KT))
                attn_finish_pair(p, po, OA_t)
                a2s.append(norm_pair(p, OA_t))
            out_proj(*pending)
            pending = (qt, a2s)
        out_proj(*pending)

    nc.compile()
    return nc


_CACHE = {}
_CACHE_LOCK = threading.Lock()


def _get_program():
    with _CACHE_LOCK:
        if "nc" not in _CACHE:
            _CACHE["nc"] = build_program()
        return _CACHE["nc"]


def _prep_inputs(q, k, v, Wq, bq, Wk, bk, Wv, bv, Wo, bo):
    """Build the 8 per-core input maps (all float32 numpy)."""
    scale = np.float32(1.0 / np.sqrt(DK))

    xs = []
    for b_i in range(B):
        xs.append((np.ascontiguousarray(q[b_i].T),
                   np.ascontiguousarray(k[b_i].T),
                   np.ascontiguousarray(v[b_i].T)))

    in_maps = []
    for c in range(N_CORES):
        b_i, g = divmod(c, HG)
        sl = slice(g * EW, (g + 1) * EW)
        wo_c = Wo[:, sl].T  # [EW, D] = [(2 pairs x 128), D]
        wo_c = np.ascontiguousarray(
            wo_c.reshape(2, 128, D).transpose(1, 0, 2))  # [128, 2, D]
        bq_c = np.ascontiguousarray(
            (bq[sl] * scale).reshape(2, 128).T)  # [128, 2]
        bk_c = np.ascontiguousarray(bk[sl].reshape(2, 128).T)
        bv_c = np.ascontiguousarray(
            np.broadcast_to(bv[sl].reshape(1, HG, DK), (128, HG, DK)))
        in_maps.append({
            "xq": xs[b_i][0], "xk": xs[b_i][1], "xv": xs[b_i][2],
            "wq": np.ascontiguousarray(Wq[sl].T * scale),
            "wk": np.ascontiguousarray(Wk[sl].T),
            "wv": np.ascontiguousarray(Wv[sl].T),
            "wo": wo_c,
            "bq2": bq_c, "bk2": bk_c, "bvr": bv_c,
        })
    return in_maps


def kernel(q, k, v, Wq, bq, Wk, bk, Wv, bv, Wo, bo):
    q = np.asarray(q, np.float32)
    k = np.asarray(k, np.float32)
    v = np.asarray(v, np.float32)
    in_maps = _prep_inputs(q, k, v,
                           np.asarray(Wq, np.float32), np.asarray(bq, np.float32),
                           np.asarray(Wk, np.float32), np.asarray(bk, np.float32),
                           np.asarray(Wv, np.float32), np.asarray(bv, np.float32),
                           np.asarray(Wo, np.float32), np.asarray(bo, np.float32))
    nc = _get_program()
    res = run_bass_kernel_spmd(nc, in_maps, core_ids=list(range(N_CORES)))
    out = np.zeros((B, S, D), np.float32)
    for c in range(N_CORES):
        b_i = c // HG
        out[b_i] += res.results[c]["outT"].T
    out += np.asarray(bo, np.float32)
    return out
